# revision 37
# baseline (speedup 1.0000x reference)
"""Distributed multi-head attention kernel for one TRN2 chip (8 NeuronCores).

Problem (B=2, N=2048, E=1024, H=16, D=64):
  q = x @ Wq.T + bq ; k = x @ Wk.T + bk ; v = x @ Wv.T + bv   (16 heads)
  energy = q @ k.T per head
  attention = softmax(energy, axis=-1) / sqrt(E)      (scale AFTER softmax)
  out = merge_heads(attention @ v) @ Wo.T + bo

Sharding: core c -> batch b = c//4, head group r = c%4 (heads 4r..4r+3).
One SPMD graph; per-core behaviour comes entirely from per-core input data.

Per core:
  phase 1: q_T/k_T in [head_dim, n] layout and v in [n, head_dim] layout,
           projected from host-pre-transposed xT and weight slices. q/k
           biases are applied per-partition during the PSUM->SBUF copy
           (ScalarE Identity with bias AP); v bias via a K=1 matmul.
  phase 2: per (token-block Q, head h): energy tiles e_T[k,q] = k_T.T @ q_T,
           exp on ScalarE (no max subtraction: |energy| < ~30), context
           ctx_T[d,q] = v_aug.T @ att accumulated on TensorE. v_aug carries a
           ones column so ctx_T row 64 is the softmax denominator; normalize
           via fast reciprocal + gpsimd partition-broadcast + multiply.
  phase 3: two AllGathers of ctx_T over the 4-core batch group (issued as
           each half of the heads completes), then every core computes ALL
           2048 tokens but only ITS 256 output columns:
           out[:, 256r:256r+256] = ctx_full @ (Wo.T/32)[:, cols] + bo[cols].
           The per-core column slice is selected by per-core INPUT data
           (the woT slice), so the graph stays fully static/SPMD and the
           head-group reduction needs no ReduceScatter at all.

Matmul operands are bf16 (PE full rate at any K); accumulation is fp32 in
PSUM. The 1/sqrt(E) scale is folded into Wo on the host.
"""

import numpy as np

B, N, E, H = 2, 2048, 1024, 16
D = E // H          # 64
N_CORES = 8
GROUPS = [[0, 1, 2, 3], [4, 5, 6, 7]]
HPC = 4             # heads per core
HD = HPC * D        # 256 head-dims per core
KC = E // 128       # 8 contraction chunks for projections
NB = N // 512       # 4 blocks of 512 (tokens)
NC16 = N // 128     # 16 chunks of 128 (keys)

_CACHE = {}


def _build():
    import os
    import concourse.bacc as bacc
    import concourse.mybir as mybir
    import concourse.tile as tile

    F32 = mybir.dt.float32
    F32R = mybir.dt.float32r
    BF16 = mybir.dt.bfloat16
    EXP = mybir.ActivationFunctionType.Exp
    IDENT = mybir.ActivationFunctionType.Identity

    # bisect flag: "full" | "noout" (skip phase 3) | "nors" (no collective)
    mode = os.environ.get("KERNEL_MODE", "full")

    nc = bacc.Bacc("TRN2", target_bir_lowering=False, debug=False,
                   num_devices=N_CORES)

    xT = nc.dram_tensor("xT", [E, N], BF16, kind="ExternalInput")
    wqT = nc.dram_tensor("wqT", [E, HD], BF16, kind="ExternalInput")
    wkT = nc.dram_tensor("wkT", [E, HD], BF16, kind="ExternalInput")
    wvT = nc.dram_tensor("wvT", [E, HD], BF16, kind="ExternalInput")
    woT = nc.dram_tensor("woT", [E, HD], BF16, kind="ExternalInput")
    bqc = nc.dram_tensor("bqc", [128, 2], F32, kind="ExternalInput")
    bkc = nc.dram_tensor("bkc", [128, 2], F32, kind="ExternalInput")
    bv = nc.dram_tensor("bv", [HD], BF16, kind="ExternalInput")
    bo4 = nc.dram_tensor("bo4", [HD], BF16, kind="ExternalInput")
    ones = nc.dram_tensor("ones", [512], BF16, kind="ExternalInput")
    onesr = nc.dram_tensor("onesr", [64], F32R, kind="ExternalInput")
    vones = nc.dram_tensor("vones", [128, NC16 * HPC], BF16,
                           kind="ExternalInput")
    out_ext = nc.dram_tensor("out", [N, HD], F32, kind="ExternalOutput")

    with tile.TileContext(nc) as tc:
        with (
            tc.tile_pool(name="w_pool", bufs=1) as w_pool,
            tc.tile_pool(name="qkv_pool", bufs=1) as qkv_pool,
            tc.tile_pool(name="dram", bufs=2, space="DRAM") as dram,
        ):
            # ---- weights / constants ---------------------------------
            wq_sb = w_pool.tile([128, KC * HD], BF16, name="wq_sb")
            wk_sb = w_pool.tile([128, KC * HD], BF16, name="wk_sb")
            wv_sb = w_pool.tile([128, KC * HD], BF16, name="wv_sb")
            wo_sb = w_pool.tile([128, KC * HD], BF16, name="wo_sb")
            bqc_sb = w_pool.tile([128, 2], F32, name="bqc_sb")
            bkc_sb = w_pool.tile([128, 2], F32, name="bkc_sb")
            bv_sb = w_pool.tile([1, HD], BF16, name="bv_sb")
            bo_sb = w_pool.tile([1, HD], BF16, name="bo_sb")
            ones_sb = w_pool.tile([1, 512], BF16, name="ones_sb")
            onesr_sb = w_pool.tile([1, 64], F32R, name="onesr_sb")
            nc.sync.dma_start(out=onesr_sb[:], in_=onesr[None, :])
            nc.sync.dma_start(out=bqc_sb[:], in_=bqc[:])
            nc.sync.dma_start(out=bkc_sb[:], in_=bkc[:])
            nc.sync.dma_start(out=bv_sb[:], in_=bv[None, :])
            nc.sync.dma_start(out=bo_sb[:], in_=bo4[None, :])
            nc.sync.dma_start(out=ones_sb[:], in_=ones[None, :])

            # q_T/k_T/ctx_T: [128, 2 blocks x N]; head h at partitions
            # 64*(h%2).., free block N*(h//2)
            q_sb = qkv_pool.tile([128, 2 * N], BF16, name="q_sb")
            k_sb = qkv_pool.tile([128, 2 * N], BF16, name="k_sb")
            ctx_sb = qkv_pool.tile([128, 2 * N], BF16, name="ctx_sb")
            # v: per 128-token chunk: 4 heads x (64 d + 1 ones col)
            v_sb = qkv_pool.tile([128, NC16 * 260], BF16, name="v_sb")
            nc.sync.dma_start(
                out=v_sb[:].rearrange("p (c h o) -> p c h o", c=NC16, h=HPC)[
                    :, :, :, 64:65
                ],
                in_=vones[:].rearrange("p (c h) -> p c h", c=NC16)[:, :, :, None],
            )

            # ---- phase 1: projections --------------------------------
            with (
                tc.tile_pool(name="x_pool", bufs=KC) as x_pool,
                tc.tile_pool(name="p1_ps", bufs=3, space="PSUM") as p1_ps,
            ):
                # interleave x / weight chunk loads so the kc=0 matmuls can
                # start after ~0.6MB instead of after all weights
                x_c = []
                for kc in range(KC):
                    ks = slice(128 * kc, 128 * (kc + 1))
                    cs = slice(HD * kc, HD * (kc + 1))
                    xt = x_pool.tile([128, N], BF16, name=f"x_c{kc}", tag="x_c")
                    xq = nc.sync if kc % 2 == 0 else nc.scalar
                    xq.dma_start(out=xt[:], in_=xT[ks, :])
                    nc.gpsimd.dma_start(out=wq_sb[:, cs], in_=wqT[ks, :])
                    nc.gpsimd.dma_start(out=wk_sb[:, cs], in_=wkT[ks, :])
                    nc.sync.dma_start(out=wv_sb[:, cs], in_=wvT[ks, :])
                    x_c.append(xt)
                # out-proj weights are needed only in phase 3
                for kc in range(KC):
                    nc.sync.dma_start(
                        out=wo_sb[:, HD * kc: HD * (kc + 1)],
                        in_=woT[128 * kc: 128 * (kc + 1), :],
                    )

                def proj_qk(w_sb, bcol_sb, dst, m, nb):
                    ps = p1_ps.tile([128, 512], F32, name=f"ps{m}{nb}", tag="p1")
                    for kc in range(KC):
                        nc.tensor.matmul(
                            ps[:],
                            w_sb[:, HD * kc + 128 * m: HD * kc + 128 * (m + 1)],
                            x_c[kc][:, 512 * nb: 512 * (nb + 1)],
                            start=(kc == 0), stop=(kc == KC - 1),
                        )
                    # copy + per-partition bias on ScalarE (idle in phase 1)
                    nc.scalar.activation(
                        dst[:, N * m + 512 * nb: N * m + 512 * (nb + 1)], ps[:],
                        IDENT, bias=bcol_sb[:, m:m + 1],
                    )

                def proj_v(nck):
                    ps = p1_ps.tile([128, HD], F32, name=f"psv{nck}", tag="p1v")
                    for kc in range(KC):
                        nc.tensor.matmul(
                            ps[:],
                            x_c[kc][:, 128 * nck: 128 * (nck + 1)],
                            wv_sb[:, HD * kc: HD * (kc + 1)],
                            start=(kc == 0), stop=False,
                        )
                    nc.tensor.matmul(
                        ps[:], ones_sb[0:1, 0:128], bv_sb[0:1, :],
                        start=False, stop=True,
                    )
                    nc.vector.tensor_copy(
                        v_sb[:, 260 * nck: 260 * nck + 260].rearrange(
                            "p (h o) -> p h o", h=HPC
                        )[:, :, 0:64],
                        ps[:].rearrange("p (h d) -> p h d", h=HPC),
                    )

                for nb in range(NB):
                    proj_qk(wk_sb, bkc_sb, k_sb, 0, nb)
                for nb in range(NB):
                    proj_qk(wq_sb, bqc_sb, q_sb, 0, nb)
                for nck in range(NC16):
                    proj_v(nck)
                for nb in range(NB):
                    proj_qk(wk_sb, bkc_sb, k_sb, 1, nb)
                for nb in range(NB):
                    proj_qk(wq_sb, bqc_sb, q_sb, 1, nb)

            # ---- phase 2: attention (hp outer, qb inner) -------------
            # The collective engine moves ~4MB/core serially at ~25GB/s, so
            # the AllGather chain must START as early as possible: graduated
            # token chunks, first one after just one attention unit.
            # (hp, qb_after, tok_lo, tok_hi):
            AG_CHUNKS = [
                (0, 0, 0, 512), (0, 1, 512, 1024), (0, 3, 1024, 2048),
                (1, 1, 0, 1024), (1, 2, 1024, 1536), (1, 3, 1536, 2048),
            ]
            ag_out = {}
            ag_stage = [None, None]
            for hp in range(2):
                # block-major staging: rows [128*qb, +128) hold token block qb,
                # so every AG chunk (= whole blocks) is a contiguous region
                ag_stage[hp] = dram.tile([512, 512], BF16, name=f"ags{hp}",
                                         tag=f"ags{hp}", bufs=1)
            for ci, (hp, qa, lo, hi) in enumerate(AG_CHUNKS):
                nblk = (hi - lo) // 512
                ag_out[ci] = dram.tile([4 * nblk * 128, 512], BF16,
                                       name=f"ago{ci}", tag=f"ago{ci}", bufs=1)
            with (
                tc.tile_pool(name="att_pool", bufs=8) as att_pool,
                tc.tile_pool(name="e_ps", bufs=3, space="PSUM") as e_ps,
                tc.tile_pool(name="c_ps", bufs=2, space="PSUM") as c_ps,
                tc.tile_pool(name="nrm_pool", bufs=3) as nrm_pool,
            ):
                for hp in range(HPC // 2):        # head pairs (2hp, 2hp+1)
                    fb = N * hp
                    for qb in range(NB):
                        qs = slice(fb + 512 * qb, fb + 512 * (qb + 1))
                        pcs = [
                            c_ps.tile([65, 512], F32, name=f"pc{qb}{hp}{t}",
                                      tag="pc")
                            for t in range(2)
                        ]
                        for kc in range(NC16):
                            # one PSUM tile holds BOTH heads' energies for this
                            # key chunk: halves unblock together, so the two
                            # row-packed matmuls truly overlap in the PE array
                            pe = e_ps.tile([128, 1024], F32,
                                           name=f"pe{qb}{hp}{kc}", tag="pe")
                            att = att_pool.tile([128, 1024], BF16,
                                                name=f"at{qb}{hp}{kc}",
                                                tag="att")
                            kcol = slice(fb + 128 * kc, fb + 128 * (kc + 1))
                            for t in range(2):
                                p0 = 64 * t
                                nc.tensor.matmul(
                                    pe[:, 512 * t: 512 * (t + 1)],
                                    k_sb[p0:p0 + 64, kcol],
                                    q_sb[p0:p0 + 64, qs],
                                    start=True, stop=True,
                                    tile_position=(p0, 0),
                                )
                            nc.scalar.activation(att[:], pe[:], EXP)
                            for t in range(2):
                                h = 2 * hp + t
                                nc.tensor.matmul(
                                    pcs[t][:],
                                    v_sb[:, 260 * kc + 65 * h:
                                         260 * kc + 65 * (h + 1)],
                                    att[:, 512 * t: 512 * (t + 1)],
                                    start=(kc == 0), stop=(kc == NC16 - 1),
                                )
                        for t in range(2):
                            p0 = 64 * t
                            pc = pcs[t]
                            den = nrm_pool.tile([1, 512], F32,
                                                name=f"dn{qb}{hp}{t}", tag="den")
                            recip = nrm_pool.tile([1, 512], F32,
                                                  name=f"rc{qb}{hp}{t}",
                                                  tag="recip")
                            recipr = nrm_pool.tile([1, 512], F32R,
                                                   name=f"rr{qb}{hp}{t}",
                                                   tag="recipr")
                            bcast = nrm_pool.tile([64, 512], F32,
                                                  name=f"bc{qb}{hp}{t}",
                                                  tag="bcast")
                            # approx-recip's bit trick is SBUF-only: stage first
                            nc.vector.tensor_copy(den[:], pc[64:65, :])
                            nc.vector.reciprocal_approx_fast(recip[:], den[:])
                            nc.vector.tensor_copy(recipr[:], recip[:])
                            # broadcast across partitions via a K=1 ones matmul
                            pb = e_ps.tile([128, 1024], F32,
                                           name=f"pb{qb}{hp}{t}", tag="pe")
                            nc.tensor.matmul(pb[0:64, 0:512], onesr_sb[:],
                                             recipr[:], start=True, stop=True)
                            nc.vector.tensor_copy(bcast[:], pb[0:64, 0:512])
                            nc.vector.tensor_mul(
                                ctx_sb[p0:p0 + 64, qs], pc[0:64, :], bcast[:]
                            )
                        # stage this block's ctx now; triggers then fire
                        # instantly at their chunk boundaries
                        if mode == "full":
                            nc.gpsimd.dma_start(
                                out=ag_stage[hp][128 * qb: 128 * (qb + 1), :],
                                in_=ctx_sb[:, fb + 512 * qb:
                                           fb + 512 * (qb + 1)],
                            )
                            for ci, (chp, qa, lo, hi) in enumerate(AG_CHUNKS):
                                if chp != hp or qa != qb:
                                    continue
                                nc.gpsimd.collective_compute(
                                    "AllGather", mybir.AluOpType.bypass,
                                    replica_groups=GROUPS,
                                    ins=[ag_stage[hp][128 * (lo // 512):
                                                      128 * (hi // 512), :]
                                         .opt()],
                                    outs=[ag_out[ci].opt()],
                                )

            # ---- phase 3: every core computes ALL tokens for ITS 256
            # output columns (selected by its woT input slice), so the
            # head-group reduction needs no ReduceScatter -----------------
            if mode != "full":
                for nt in range(NC16):
                    nc.sync.dma_start(
                        out=out_ext[128 * nt: 128 * (nt + 1), :],
                        in_=ctx_sb[:, 0:512].bitcast(F32),
                    )
            else:
                with (
                    tc.tile_pool(name="cf_pool", bufs=1) as cf_pool,
                    tc.tile_pool(name="o_ps", bufs=3, space="PSUM") as o_ps,
                    tc.tile_pool(name="osb_pool", bufs=6) as osb_pool,
                ):
                    # cf: [128, 8 kc x N]; kc = (hp, g): global head-dim rows
                    # [256g + 128hp .. +128) -> host permutes woT rows to match
                    cf_sb = cf_pool.tile([128, KC * N], BF16, name="cf_sb")
                    # split across two DMA queues; earliest chunks first
                    for ci, (hp, qa, lo, hi) in sorted(
                        enumerate(AG_CHUNKS), key=lambda e: (e[1][1], e[1][0])
                    ):
                        eng = nc.sync if hp == 0 else nc.gpsimd
                        nblk = (hi - lo) // 512
                        for g in range(4):
                            for bl in range(nblk):
                                row = (g * nblk + bl) * 128
                                col = N * (4 * hp + g) + lo + 512 * bl
                                eng.dma_start(
                                    out=cf_sb[:, col: col + 512],
                                    in_=ag_out[ci][row: row + 128, :],
                                )
                    for nt in range(NC16):        # 16 token tiles of 128
                        tok = 128 * nt
                        po = o_ps.tile([128, HD], F32, name=f"po{nt}",
                                       tag="po")
                        for kc in range(KC):
                            nc.tensor.matmul(
                                po[:],
                                cf_sb[:, N * kc + tok: N * kc + tok + 128],
                                wo_sb[:, HD * kc: HD * (kc + 1)],
                                start=(kc == 0), stop=False,
                            )
                        nc.tensor.matmul(
                            po[:], ones_sb[0:1, 0:128], bo_sb[0:1, :],
                            start=False, stop=True,
                        )
                        osb = osb_pool.tile([128, HD], F32, name=f"ob{nt}",
                                            tag="osb")
                        nc.vector.tensor_copy(osb[:], po[:])
                        nc.scalar.dma_start(
                            out=out_ext[tok: tok + 128, :], in_=osb[:]
                        )

    nc.compile()
    return nc


def _get_compiled():
    if "nc" not in _CACHE:
        _CACHE["nc"] = _build()
    return _CACHE["nc"]


def _make_in_maps(x, Wq, bq, Wk, bk, Wv, bv, Wo, bo):
    import ml_dtypes
    bf = ml_dtypes.bfloat16
    f = np.float32
    x = np.asarray(x, f)
    WqT = np.ascontiguousarray(np.asarray(Wq, f).T)
    WkT = np.ascontiguousarray(np.asarray(Wk, f).T)
    WvT = np.ascontiguousarray(np.asarray(Wv, f).T)
    WoTs = np.ascontiguousarray(np.asarray(Wo, f).T / np.sqrt(np.float32(E)))
    # row order must match the AllGather layout: chunk kc=(hp,g) holds global
    # head-dim rows [256g + 128hp, +128)
    perm = np.concatenate([
        np.arange(256 * g + 128 * hp, 256 * g + 128 * hp + 128)
        for hp in range(2) for g in range(4)
    ])
    WoTp = np.ascontiguousarray(WoTs[perm, :])
    bq = np.asarray(bq, f); bk = np.asarray(bk, f)
    bv = np.asarray(bv, f); bo = np.asarray(bo, f)
    xTs = [np.ascontiguousarray(x[b].T).astype(bf) for b in range(B)]
    ones = np.ones(512, bf)
    onesr = np.ones(64, f)
    vones = np.ones((128, NC16 * HPC), bf)
    in_maps = []
    for c in range(N_CORES):
        b, r = divmod(c, 4)
        hd = slice(HD * r, HD * (r + 1))
        in_maps.append({
            "xT": xTs[b],
            "wqT": np.ascontiguousarray(WqT[:, hd]).astype(bf),
            "wkT": np.ascontiguousarray(WkT[:, hd]).astype(bf),
            "wvT": np.ascontiguousarray(WvT[:, hd]).astype(bf),
            "woT": np.ascontiguousarray(WoTp[:, hd]).astype(bf),
            "bqc": np.ascontiguousarray(bq[hd].reshape(2, 128).T),
            "bkc": np.ascontiguousarray(bk[hd].reshape(2, 128).T),
            "bv": np.ascontiguousarray(bv[hd]).astype(bf),
            "bo4": np.ascontiguousarray(bo[hd]).astype(bf),
            "ones": ones,
            "onesr": onesr,
            "vones": vones,
        })
    return in_maps


def _assemble(results):
    out = np.empty((B, N, E), np.float32)
    for c in range(N_CORES):
        b, r = divmod(c, 4)
        out[b, :, HD * r: HD * (r + 1)] = results[c]["out"]   # [N, 256]
    return out


def run_spmd(inputs, trace=False):
    """Returns (full_output, BassKernelResults)."""
    from concourse.bass_utils import run_bass_kernel_spmd
    nc = _get_compiled()
    in_maps = _make_in_maps(**inputs)
    res = run_bass_kernel_spmd(nc, in_maps, core_ids=list(range(N_CORES)),
                               trace=trace)
    return _assemble(res.results), res


def _subproc_main(in_path, out_path):
    data = np.load(in_path)
    out, _ = run_spmd({k: data[k] for k in data.files}, trace=False)
    np.save(out_path, out)


def kernel(**inputs):
    """Runs the SPMD kernel; retries in a fresh process if the accelerator
    session is left in a bad state by a previous crashed run (transient)."""
    try:
        out, _ = run_spmd(inputs, trace=False)
        return out
    except Exception as e:      # noqa: BLE001 - device-state errors vary
        import os
        import subprocess
        import sys
        import tempfile
        import time
        err = e
        here = os.path.abspath(os.path.dirname(__file__))
        with tempfile.TemporaryDirectory() as td:
            in_path = os.path.join(td, "in.npz")
            out_path = os.path.join(td, "out.npy")
            np.savez(in_path,
                     **{k: np.asarray(v, np.float32)
                        for k, v in inputs.items()})
            code = (f"import sys; sys.path.insert(0, {here!r}); "
                    f"import kernel; "
                    f"kernel._subproc_main({in_path!r}, {out_path!r})")
            for attempt in range(3):
                time.sleep(15)
                r = subprocess.run([sys.executable, "-c", code],
                                   capture_output=True, text=True)
                if r.returncode == 0 and os.path.exists(out_path):
                    return np.load(out_path)
                err = RuntimeError(
                    f"subprocess retry {attempt} failed:\n{r.stderr[-2000:]}")
        raise err


# revision 38
# speedup vs baseline: 1.0177x; 1.0177x over previous
"""Distributed multi-head attention kernel for one TRN2 chip (8 NeuronCores).

Problem (B=2, N=2048, E=1024, H=16, D=64):
  q = x @ Wq.T + bq ; k = x @ Wk.T + bk ; v = x @ Wv.T + bv   (16 heads)
  energy = q @ k.T per head
  attention = softmax(energy, axis=-1) / sqrt(E)      (scale AFTER softmax)
  out = merge_heads(attention @ v) @ Wo.T + bo

Sharding: core c -> batch b = c//4, head group r = c%4 (heads 4r..4r+3).
One SPMD graph; per-core behaviour comes entirely from per-core input data.

Per core:
  phase 1: q_T/k_T in [head_dim, n] layout and v in [n, head_dim] layout,
           projected from host-pre-transposed xT and weight slices. q/k
           biases are applied per-partition during the PSUM->SBUF copy
           (ScalarE Identity with bias AP); v bias via a K=1 matmul.
  phase 2: per (token-block Q, head h): energy tiles e_T[k,q] = k_T.T @ q_T,
           exp on ScalarE (no max subtraction: |energy| < ~30), context
           ctx_T[d,q] = v_aug.T @ att accumulated on TensorE. v_aug carries a
           ones column so ctx_T row 64 is the softmax denominator; normalize
           via fast reciprocal + gpsimd partition-broadcast + multiply.
  phase 3: two AllGathers of ctx_T over the 4-core batch group (issued as
           each half of the heads completes), then every core computes ALL
           2048 tokens but only ITS 256 output columns:
           out[:, 256r:256r+256] = ctx_full @ (Wo.T/32)[:, cols] + bo[cols].
           The per-core column slice is selected by per-core INPUT data
           (the woT slice), so the graph stays fully static/SPMD and the
           head-group reduction needs no ReduceScatter at all.

Matmul operands are bf16 (PE full rate at any K); accumulation is fp32 in
PSUM. The 1/sqrt(E) scale is folded into Wo on the host.
"""

import numpy as np

B, N, E, H = 2, 2048, 1024, 16
D = E // H          # 64
N_CORES = 8
GROUPS = [[0, 1, 2, 3], [4, 5, 6, 7]]
HPC = 4             # heads per core
HD = HPC * D        # 256 head-dims per core
KC = E // 128       # 8 contraction chunks for projections
NB = N // 512       # 4 blocks of 512 (tokens)
NC16 = N // 128     # 16 chunks of 128 (keys)

_CACHE = {}


def _build():
    import os
    import concourse.bacc as bacc
    import concourse.mybir as mybir
    import concourse.tile as tile

    F32 = mybir.dt.float32
    F32R = mybir.dt.float32r
    BF16 = mybir.dt.bfloat16
    EXP = mybir.ActivationFunctionType.Exp
    IDENT = mybir.ActivationFunctionType.Identity

    # bisect flag: "full" | "noout" (skip phase 3) | "nors" (no collective)
    mode = os.environ.get("KERNEL_MODE", "full")

    nc = bacc.Bacc("TRN2", target_bir_lowering=False, debug=False,
                   num_devices=N_CORES)

    xT = nc.dram_tensor("xT", [E, N], BF16, kind="ExternalInput")
    wqT = nc.dram_tensor("wqT", [E, HD], BF16, kind="ExternalInput")
    wkT = nc.dram_tensor("wkT", [E, HD], BF16, kind="ExternalInput")
    wvT = nc.dram_tensor("wvT", [E, HD], BF16, kind="ExternalInput")
    woT = nc.dram_tensor("woT", [E, HD], BF16, kind="ExternalInput")
    bqc = nc.dram_tensor("bqc", [128, 2], F32, kind="ExternalInput")
    bkc = nc.dram_tensor("bkc", [128, 2], F32, kind="ExternalInput")
    bv = nc.dram_tensor("bv", [HD], BF16, kind="ExternalInput")
    bo4 = nc.dram_tensor("bo4", [HD], BF16, kind="ExternalInput")
    ones = nc.dram_tensor("ones", [512], BF16, kind="ExternalInput")
    onesr = nc.dram_tensor("onesr", [64], F32R, kind="ExternalInput")
    vones = nc.dram_tensor("vones", [128, NC16 * HPC], BF16,
                           kind="ExternalInput")
    out_ext = nc.dram_tensor("out", [N, HD], F32, kind="ExternalOutput")

    with tile.TileContext(nc) as tc:
        with (
            tc.tile_pool(name="w_pool", bufs=1) as w_pool,
            tc.tile_pool(name="qkv_pool", bufs=1) as qkv_pool,
            tc.tile_pool(name="dram", bufs=2, space="DRAM") as dram,
        ):
            # ---- weights / constants ---------------------------------
            wq_sb = w_pool.tile([128, KC * HD], BF16, name="wq_sb")
            wk_sb = w_pool.tile([128, KC * HD], BF16, name="wk_sb")
            wv_sb = w_pool.tile([128, KC * HD], BF16, name="wv_sb")
            wo_sb = w_pool.tile([128, KC * HD], BF16, name="wo_sb")
            bqc_sb = w_pool.tile([128, 2], F32, name="bqc_sb")
            bkc_sb = w_pool.tile([128, 2], F32, name="bkc_sb")
            bv_sb = w_pool.tile([1, HD], BF16, name="bv_sb")
            bo_sb = w_pool.tile([1, HD], BF16, name="bo_sb")
            ones_sb = w_pool.tile([1, 512], BF16, name="ones_sb")
            onesr_sb = w_pool.tile([1, 64], F32R, name="onesr_sb")
            nc.sync.dma_start(out=onesr_sb[:], in_=onesr[None, :])
            nc.sync.dma_start(out=bqc_sb[:], in_=bqc[:])
            nc.sync.dma_start(out=bkc_sb[:], in_=bkc[:])
            nc.sync.dma_start(out=bv_sb[:], in_=bv[None, :])
            nc.sync.dma_start(out=bo_sb[:], in_=bo4[None, :])
            nc.sync.dma_start(out=ones_sb[:], in_=ones[None, :])

            # q_T/k_T/ctx_T: [128, 2 blocks x N]; head h at partitions
            # 64*(h%2).., free block N*(h//2)
            q_sb = qkv_pool.tile([128, 2 * N], BF16, name="q_sb")
            k_sb = qkv_pool.tile([128, 2 * N], BF16, name="k_sb")
            ctx_sb = qkv_pool.tile([128, 2 * N], BF16, name="ctx_sb")
            # v: per 128-token chunk: 4 heads x (64 d + 1 ones col)
            v_sb = qkv_pool.tile([128, NC16 * 260], BF16, name="v_sb")
            nc.sync.dma_start(
                out=v_sb[:].rearrange("p (c h o) -> p c h o", c=NC16, h=HPC)[
                    :, :, :, 64:65
                ],
                in_=vones[:].rearrange("p (c h) -> p c h", c=NC16)[:, :, :, None],
            )

            # ---- phase 1: projections --------------------------------
            with (
                tc.tile_pool(name="x_pool", bufs=KC) as x_pool,
                tc.tile_pool(name="p1_ps", bufs=3, space="PSUM") as p1_ps,
            ):
                # interleave x / weight chunk loads so the kc=0 matmuls can
                # start after ~0.6MB instead of after all weights
                x_c = []
                for kc in range(KC):
                    ks = slice(128 * kc, 128 * (kc + 1))
                    cs = slice(HD * kc, HD * (kc + 1))
                    xt = x_pool.tile([128, N], BF16, name=f"x_c{kc}", tag="x_c")
                    xq = nc.sync if kc % 2 == 0 else nc.scalar
                    xq.dma_start(out=xt[:], in_=xT[ks, :])
                    nc.gpsimd.dma_start(out=wq_sb[:, cs], in_=wqT[ks, :])
                    nc.gpsimd.dma_start(out=wk_sb[:, cs], in_=wkT[ks, :])
                    nc.sync.dma_start(out=wv_sb[:, cs], in_=wvT[ks, :])
                    x_c.append(xt)
                # out-proj weights are needed only in phase 3
                for kc in range(KC):
                    nc.sync.dma_start(
                        out=wo_sb[:, HD * kc: HD * (kc + 1)],
                        in_=woT[128 * kc: 128 * (kc + 1), :],
                    )

                def proj_qk(w_sb, bcol_sb, dst, m, nb):
                    ps = p1_ps.tile([128, 512], F32, name=f"ps{m}{nb}", tag="p1")
                    for kc in range(KC):
                        nc.tensor.matmul(
                            ps[:],
                            w_sb[:, HD * kc + 128 * m: HD * kc + 128 * (m + 1)],
                            x_c[kc][:, 512 * nb: 512 * (nb + 1)],
                            start=(kc == 0), stop=(kc == KC - 1),
                        )
                    # copy + per-partition bias on VectorE (keeps the ACT
                    # queue clear for the exp stream)
                    nc.vector.tensor_scalar_add(
                        dst[:, N * m + 512 * nb: N * m + 512 * (nb + 1)], ps[:],
                        bcol_sb[:, m:m + 1],
                    )

                def proj_v(nck):
                    ps = p1_ps.tile([128, HD], F32, name=f"psv{nck}", tag="p1v")
                    for kc in range(KC):
                        nc.tensor.matmul(
                            ps[:],
                            x_c[kc][:, 128 * nck: 128 * (nck + 1)],
                            wv_sb[:, HD * kc: HD * (kc + 1)],
                            start=(kc == 0), stop=False,
                        )
                    nc.tensor.matmul(
                        ps[:], ones_sb[0:1, 0:128], bv_sb[0:1, :],
                        start=False, stop=True,
                    )
                    nc.vector.tensor_copy(
                        v_sb[:, 260 * nck: 260 * nck + 260].rearrange(
                            "p (h o) -> p h o", h=HPC
                        )[:, :, 0:64],
                        ps[:].rearrange("p (h d) -> p h d", h=HPC),
                    )

                proj_qk(wk_sb, bkc_sb, k_sb, 0, 0)
                proj_qk(wq_sb, bqc_sb, q_sb, 0, 0)
                for nb in range(1, NB):
                    proj_qk(wk_sb, bkc_sb, k_sb, 0, nb)
                for nb in range(1, NB):
                    proj_qk(wq_sb, bqc_sb, q_sb, 0, nb)
                for nck in range(NC16):
                    proj_v(nck)
                for nb in range(NB):
                    proj_qk(wk_sb, bkc_sb, k_sb, 1, nb)
                for nb in range(NB):
                    proj_qk(wq_sb, bqc_sb, q_sb, 1, nb)

            # ---- phase 2: attention (hp outer, qb inner) -------------
            # The collective engine moves ~4MB/core serially at ~25GB/s, so
            # the AllGather chain must START as early as possible: graduated
            # token chunks, first one after just one attention unit.
            # (hp, qb_after, tok_lo, tok_hi):
            AG_CHUNKS = [
                (0, 0, 0, 512), (0, 1, 512, 1024), (0, 3, 1024, 2048),
                (1, 1, 0, 1024), (1, 2, 1024, 1536), (1, 3, 1536, 2048),
            ]
            ag_out = {}
            ag_stage = [None, None]
            for hp in range(2):
                # block-major staging: rows [128*qb, +128) hold token block qb,
                # so every AG chunk (= whole blocks) is a contiguous region
                ag_stage[hp] = dram.tile([512, 512], BF16, name=f"ags{hp}",
                                         tag=f"ags{hp}", bufs=1)
            for ci, (hp, qa, lo, hi) in enumerate(AG_CHUNKS):
                nblk = (hi - lo) // 512
                ag_out[ci] = dram.tile([4 * nblk * 128, 512], BF16,
                                       name=f"ago{ci}", tag=f"ago{ci}", bufs=1)
            with (
                tc.tile_pool(name="att_pool", bufs=8) as att_pool,
                tc.tile_pool(name="e_ps", bufs=3, space="PSUM") as e_ps,
                tc.tile_pool(name="c_ps", bufs=2, space="PSUM") as c_ps,
                tc.tile_pool(name="nrm_pool", bufs=3) as nrm_pool,
            ):
                for hp in range(HPC // 2):        # head pairs (2hp, 2hp+1)
                    fb = N * hp
                    for qb in range(NB):
                        qs = slice(fb + 512 * qb, fb + 512 * (qb + 1))
                        pcs = [
                            c_ps.tile([65, 512], F32, name=f"pc{qb}{hp}{t}",
                                      tag="pc")
                            for t in range(2)
                        ]
                        for kc in range(NC16):
                            # one PSUM tile holds BOTH heads' energies for this
                            # key chunk: halves unblock together, so the two
                            # row-packed matmuls truly overlap in the PE array
                            pe = e_ps.tile([128, 1024], F32,
                                           name=f"pe{qb}{hp}{kc}", tag="pe")
                            att = att_pool.tile([128, 1024], BF16,
                                                name=f"at{qb}{hp}{kc}",
                                                tag="att")
                            kcol = slice(fb + 128 * kc, fb + 128 * (kc + 1))
                            for t in range(2):
                                p0 = 64 * t
                                nc.tensor.matmul(
                                    pe[:, 512 * t: 512 * (t + 1)],
                                    k_sb[p0:p0 + 64, kcol],
                                    q_sb[p0:p0 + 64, qs],
                                    start=True, stop=True,
                                    tile_position=(p0, 0),
                                )
                            nc.scalar.activation(att[:], pe[:], EXP)
                            for t in range(2):
                                h = 2 * hp + t
                                nc.tensor.matmul(
                                    pcs[t][:],
                                    v_sb[:, 260 * kc + 65 * h:
                                         260 * kc + 65 * (h + 1)],
                                    att[:, 512 * t: 512 * (t + 1)],
                                    start=(kc == 0), stop=(kc == NC16 - 1),
                                )
                        for t in range(2):
                            p0 = 64 * t
                            pc = pcs[t]
                            den = nrm_pool.tile([1, 512], F32,
                                                name=f"dn{qb}{hp}{t}", tag="den")
                            recip = nrm_pool.tile([1, 512], F32,
                                                  name=f"rc{qb}{hp}{t}",
                                                  tag="recip")
                            recipr = nrm_pool.tile([1, 512], F32R,
                                                   name=f"rr{qb}{hp}{t}",
                                                   tag="recipr")
                            bcast = nrm_pool.tile([64, 512], F32,
                                                  name=f"bc{qb}{hp}{t}",
                                                  tag="bcast")
                            # approx-recip's bit trick is SBUF-only: stage first
                            nc.vector.tensor_copy(den[:], pc[64:65, :])
                            nc.vector.reciprocal_approx_fast(recip[:], den[:])
                            nc.vector.tensor_copy(recipr[:], recip[:])
                            # broadcast across partitions via a K=1 ones matmul
                            pb = e_ps.tile([128, 1024], F32,
                                           name=f"pb{qb}{hp}{t}", tag="pe")
                            nc.tensor.matmul(pb[0:64, 0:512], onesr_sb[:],
                                             recipr[:], start=True, stop=True)
                            nc.vector.tensor_copy(bcast[:], pb[0:64, 0:512])
                            nc.vector.tensor_mul(
                                ctx_sb[p0:p0 + 64, qs], pc[0:64, :], bcast[:]
                            )
                        # stage this block's ctx now; triggers then fire
                        # instantly at their chunk boundaries
                        if mode == "full":
                            nc.gpsimd.dma_start(
                                out=ag_stage[hp][128 * qb: 128 * (qb + 1), :],
                                in_=ctx_sb[:, fb + 512 * qb:
                                           fb + 512 * (qb + 1)],
                            )
                            for ci, (chp, qa, lo, hi) in enumerate(AG_CHUNKS):
                                if chp != hp or qa != qb:
                                    continue
                                nc.gpsimd.collective_compute(
                                    "AllGather", mybir.AluOpType.bypass,
                                    replica_groups=GROUPS,
                                    ins=[ag_stage[hp][128 * (lo // 512):
                                                      128 * (hi // 512), :]
                                         .opt()],
                                    outs=[ag_out[ci].opt()],
                                )

            # ---- phase 3: every core computes ALL tokens for ITS 256
            # output columns (selected by its woT input slice), so the
            # head-group reduction needs no ReduceScatter -----------------
            if mode != "full":
                for nt in range(NC16):
                    nc.sync.dma_start(
                        out=out_ext[128 * nt: 128 * (nt + 1), :],
                        in_=ctx_sb[:, 0:512].bitcast(F32),
                    )
            else:
                with (
                    tc.tile_pool(name="cf_pool", bufs=1) as cf_pool,
                    tc.tile_pool(name="o_ps", bufs=3, space="PSUM") as o_ps,
                    tc.tile_pool(name="osb_pool", bufs=6) as osb_pool,
                ):
                    # cf: [128, 8 kc x N]; kc = (hp, g): global head-dim rows
                    # [256g + 128hp .. +128) -> host permutes woT rows to match
                    cf_sb = cf_pool.tile([128, KC * N], BF16, name="cf_sb")
                    # split across two DMA queues; earliest chunks first
                    for ci, (hp, qa, lo, hi) in sorted(
                        enumerate(AG_CHUNKS), key=lambda e: (e[1][1], e[1][0])
                    ):
                        eng = nc.sync if hp == 0 else nc.gpsimd
                        nblk = (hi - lo) // 512
                        for g in range(4):
                            for bl in range(nblk):
                                row = (g * nblk + bl) * 128
                                col = N * (4 * hp + g) + lo + 512 * bl
                                eng.dma_start(
                                    out=cf_sb[:, col: col + 512],
                                    in_=ag_out[ci][row: row + 128, :],
                                )
                    for nt in range(NC16):        # 16 token tiles of 128
                        tok = 128 * nt
                        po = o_ps.tile([128, HD], F32, name=f"po{nt}",
                                       tag="po")
                        for kc in range(KC):
                            nc.tensor.matmul(
                                po[:],
                                cf_sb[:, N * kc + tok: N * kc + tok + 128],
                                wo_sb[:, HD * kc: HD * (kc + 1)],
                                start=(kc == 0), stop=False,
                            )
                        nc.tensor.matmul(
                            po[:], ones_sb[0:1, 0:128], bo_sb[0:1, :],
                            start=False, stop=True,
                        )
                        osb = osb_pool.tile([128, HD], F32, name=f"ob{nt}",
                                            tag="osb")
                        nc.vector.tensor_copy(osb[:], po[:])
                        nc.scalar.dma_start(
                            out=out_ext[tok: tok + 128, :], in_=osb[:]
                        )

    nc.compile()
    return nc


def _get_compiled():
    if "nc" not in _CACHE:
        _CACHE["nc"] = _build()
    return _CACHE["nc"]


def _make_in_maps(x, Wq, bq, Wk, bk, Wv, bv, Wo, bo):
    import ml_dtypes
    bf = ml_dtypes.bfloat16
    f = np.float32
    x = np.asarray(x, f)
    WqT = np.ascontiguousarray(np.asarray(Wq, f).T)
    WkT = np.ascontiguousarray(np.asarray(Wk, f).T)
    WvT = np.ascontiguousarray(np.asarray(Wv, f).T)
    WoTs = np.ascontiguousarray(np.asarray(Wo, f).T / np.sqrt(np.float32(E)))
    # row order must match the AllGather layout: chunk kc=(hp,g) holds global
    # head-dim rows [256g + 128hp, +128)
    perm = np.concatenate([
        np.arange(256 * g + 128 * hp, 256 * g + 128 * hp + 128)
        for hp in range(2) for g in range(4)
    ])
    WoTp = np.ascontiguousarray(WoTs[perm, :])
    bq = np.asarray(bq, f); bk = np.asarray(bk, f)
    bv = np.asarray(bv, f); bo = np.asarray(bo, f)
    xTs = [np.ascontiguousarray(x[b].T).astype(bf) for b in range(B)]
    ones = np.ones(512, bf)
    onesr = np.ones(64, f)
    vones = np.ones((128, NC16 * HPC), bf)
    in_maps = []
    for c in range(N_CORES):
        b, r = divmod(c, 4)
        hd = slice(HD * r, HD * (r + 1))
        in_maps.append({
            "xT": xTs[b],
            "wqT": np.ascontiguousarray(WqT[:, hd]).astype(bf),
            "wkT": np.ascontiguousarray(WkT[:, hd]).astype(bf),
            "wvT": np.ascontiguousarray(WvT[:, hd]).astype(bf),
            "woT": np.ascontiguousarray(WoTp[:, hd]).astype(bf),
            "bqc": np.ascontiguousarray(bq[hd].reshape(2, 128).T),
            "bkc": np.ascontiguousarray(bk[hd].reshape(2, 128).T),
            "bv": np.ascontiguousarray(bv[hd]).astype(bf),
            "bo4": np.ascontiguousarray(bo[hd]).astype(bf),
            "ones": ones,
            "onesr": onesr,
            "vones": vones,
        })
    return in_maps


def _assemble(results):
    out = np.empty((B, N, E), np.float32)
    for c in range(N_CORES):
        b, r = divmod(c, 4)
        out[b, :, HD * r: HD * (r + 1)] = results[c]["out"]   # [N, 256]
    return out


def run_spmd(inputs, trace=False):
    """Returns (full_output, BassKernelResults)."""
    from concourse.bass_utils import run_bass_kernel_spmd
    nc = _get_compiled()
    in_maps = _make_in_maps(**inputs)
    res = run_bass_kernel_spmd(nc, in_maps, core_ids=list(range(N_CORES)),
                               trace=trace)
    return _assemble(res.results), res


def _subproc_main(in_path, out_path):
    data = np.load(in_path)
    out, _ = run_spmd({k: data[k] for k in data.files}, trace=False)
    np.save(out_path, out)


def kernel(**inputs):
    """Runs the SPMD kernel; retries in a fresh process if the accelerator
    session is left in a bad state by a previous crashed run (transient)."""
    try:
        out, _ = run_spmd(inputs, trace=False)
        return out
    except Exception as e:      # noqa: BLE001 - device-state errors vary
        import os
        import subprocess
        import sys
        import tempfile
        import time
        err = e
        here = os.path.abspath(os.path.dirname(__file__))
        with tempfile.TemporaryDirectory() as td:
            in_path = os.path.join(td, "in.npz")
            out_path = os.path.join(td, "out.npy")
            np.savez(in_path,
                     **{k: np.asarray(v, np.float32)
                        for k, v in inputs.items()})
            code = (f"import sys; sys.path.insert(0, {here!r}); "
                    f"import kernel; "
                    f"kernel._subproc_main({in_path!r}, {out_path!r})")
            for attempt in range(3):
                time.sleep(15)
                r = subprocess.run([sys.executable, "-c", code],
                                   capture_output=True, text=True)
                if r.returncode == 0 and os.path.exists(out_path):
                    return np.load(out_path)
                err = RuntimeError(
                    f"subprocess retry {attempt} failed:\n{r.stderr[-2000:]}")
        raise err
